# revision 14
# baseline (speedup 1.0000x reference)
"""Local (causal) attention block on 8 TRN2 NeuronCores.

Reference computation (B=2, T=2048, C=1024, H=16, D=64):
    q,k,v = x@Wq.T, x@Wk.T, x@Wv.T          (per-head D=64)
    att   = softmax(causal_mask(q k^T / sqrt(D)))
    out   = (att v) @ Wo.T
(The reference's "window" band mask reduces exactly to the plain strict
causal mask, so this is full causal attention.)

Sharding (SPMD-uniform across the 8 cores):
  core c: batch b = c//4, head-group g = c%4 (heads 4g..4g+3),
  output-channel group g (columns 256g..256g+255).
  - QKV projections head-sharded: each core computes q^T,K^T,V for its 4
    heads, all 2048 positions (f32r matmuls on f32 inputs).
  - Attention: full causal for its 4 heads (S^T layout: kv on partitions,
    q on free axis; exp on ScalarE; rowsum via a ones-column appended to V;
    normalization via gpsimd partition-broadcast of 1/rowsum).
  - O^T (bf16) exchanged between the 4 cores of a batch with an AllGather.
  - Output projection oc-sharded: each core computes out[:, 256g:256g+256]
    for the whole batch (bf16 matmuls).
Host side only shards inputs and concatenates the per-core output slices.
"""

import sys

for _p in ("/opt/trn_rl_repo",):
    if _p not in sys.path:
        sys.path.append(_p)

import numpy as np

import concourse.bass as bass
import concourse.mybir as mybir
import concourse.tile as tile
from concourse import bacc
from concourse.bass import ts
from concourse.bass_utils import run_bass_kernel_spmd

B, T, C = 2, 2048, 1024
H, D = 16, 64
SCALE = 1.0 / np.sqrt(D)
N_CORES = 8
HPC = H // 4          # heads per core = 4
COC = C // 4          # channels per core = 256
F32 = mybir.dt.float32
BF16 = mybir.dt.bfloat16
F32R = mybir.dt.float32r
NEG = -1.0e5          # additive mask value (pre-scale)


def r(ap):
    """view an f32 AP as f32r for full-rate fp32 matmul"""
    return ap.bitcast(F32R)


def self_attention(nc, psum_s, psum_o, work, qT_sb, kT_sb, v_sb, masks, otall):
    """Causal attention for this core's 4 heads, S^T layout.

    Heads are processed in row-packed pairs (each head's d=64 occupies
    PE rows 0-63 / 64-127 via tile_position) so the two QK^T matmuls of a
    pair run concurrently.
    """
    NQC = T // 512
    for pair in range(HPC // 2):
        h0, h1 = 2 * pair, 2 * pair + 1
        for qc in range(NQC):
            nkv = 4 * (qc + 1)
            ot_ps = [
                psum_o.tile([D + 1, 512], F32, tag=f"ot{i}", name=f"ot{i}")
                for i in (0, 1)
            ]
            for k in range(nkv):
                s_ps = [
                    psum_s.tile([128, 512], F32, tag=f"s{i}", name=f"s{i}")
                    for i in (0, 1)
                ]
                for i, h in ((0, h0), (1, h1)):
                    bp = 64 * (h % 2)
                    nc.tensor.matmul(
                        s_ps[i][:],
                        kT_sb[bp : bp + 64, h // 2, ts(k, 128)],
                        qT_sb[bp : bp + 64, h // 2, ts(qc, 512)],
                        start=True,
                        stop=True,
                        tile_position=(bp, 0),
                    )
                m = k - 4 * qc
                for i, h in ((0, h0), (1, h1)):
                    pt = work.tile([128, 512], BF16, tag="pt")
                    if m >= 0:  # diagonal tile: mask before exp
                        sm = work.tile([128, 512], F32, tag="sm")
                        nc.vector.tensor_add(sm[:], s_ps[i][:], masks[:, m, :])
                        src = sm
                    else:
                        src = s_ps[i]
                    nc.scalar.activation(
                        pt[:],
                        src[:],
                        mybir.ActivationFunctionType.Exp,
                        scale=float(SCALE),
                    )
                    nc.tensor.matmul(
                        ot_ps[i][:],
                        v_sb[:, k, h, :],
                        pt[:],
                        start=(k == 0),
                        stop=(k == nkv - 1),
                    )
            for i, h in ((0, h0), (1, h1)):
                recip = work.tile([1, 512], F32, tag="recip")
                bcast = work.tile([64, 512], F32, tag="bcast")
                nc.vector.reciprocal(recip[:], ot_ps[i][D : D + 1, :])
                nc.gpsimd.partition_broadcast(bcast[:], recip[:])
                nc.vector.tensor_mul(
                    otall[64 * (h % 2) : 64 * (h % 2) + 64, h // 2, ts(qc, 512)],
                    ot_ps[i][0:D, :],
                    bcast[:],
                )


def build_nc():
    nc = bacc.Bacc(
        "TRN2",
        target_bir_lowering=False,
        debug=False,
        num_devices=N_CORES,
    )
    xT_d = nc.dram_tensor("xT", [C, T], F32, kind="ExternalInput").ap()
    wqT_d = nc.dram_tensor("wqT", [C, COC], F32, kind="ExternalInput").ap()
    wkT_d = nc.dram_tensor("wkT", [C, COC], F32, kind="ExternalInput").ap()
    wvT_d = nc.dram_tensor("wvT", [C, COC], F32, kind="ExternalInput").ap()
    woT_d = nc.dram_tensor("woT", [C, COC], F32, kind="ExternalInput").ap()
    out_d = nc.dram_tensor("out", [T, COC], F32, kind="ExternalOutput").ap()

    NQC = T // 512     # 4 q-chunks of 512
    NKT = T // 128     # 16 kv tiles of 128
    NCT = C // 128     # 8 contraction tiles

    with tile.TileContext(nc) as tc:
        with (
            tc.tile_pool(name="main", bufs=1) as main,
            tc.tile_pool(name="work", bufs=4) as work,
            tc.tile_pool(name="dram", bufs=1, space="DRAM") as dram,
        ):
            # ---- long-lived SBUF tensors ----
            qT_sb = main.tile([128, 2, T], BF16)         # [co 256, t]
            kT_sb = main.tile([128, 2, T], BF16)
            v_sb = main.tile([128, NKT, HPC, D + 1], BF16)  # V + ones col
            otall = main.tile([128, 2, T], BF16)         # own O^T (normalized)
            otfull = main.tile([128, NCT, T], BF16)      # gathered O^T, all C
            woT_bf = main.tile([128, NCT, COC], BF16)
            masks = main.tile([128, 4, 512], F32)

            # ---- phase 1: projections (f32r), x^T streamed in t-chunks ----
            xT_r = xT_d.rearrange("(a p) t -> p a t", p=128)
            with (
                tc.tile_pool(name="p1w", bufs=1) as p1w,
                tc.tile_pool(name="p1ws", bufs=2) as p1ws,
                tc.tile_pool(name="p1x", bufs=2) as p1x,
                tc.tile_pool(name="p1psum", bufs=2, space="PSUM") as psum_p1,
            ):
                wq_sb = p1w.tile([128, NCT, COC], BF16)
                wk_sb = p1w.tile([128, NCT, COC], BF16)
                wv_sb = p1w.tile([128, NCT, COC], BF16)
                for w_sb, w_d in ((wq_sb, wqT_d), (wk_sb, wkT_d), (wv_sb, wvT_d)):
                    wst = p1ws.tile([128, NCT, COC], F32, tag="wst")
                    nc.sync.dma_start(
                        out=wst[:], in_=w_d.rearrange("(a p) t -> p a t", p=128)
                    )
                    nc.vector.tensor_copy(w_sb[:], wst[:])

                for tj in range(NQC):
                    xch = p1x.tile([128, NCT, 512], F32, tag="xch")
                    nc.sync.dma_start(out=xch[:], in_=xT_r[:, :, ts(tj, 512)])
                    xbf = p1x.tile([128, NCT, 512], BF16, tag="xbf")
                    nc.vector.tensor_copy(xbf[:], xch[:])

                    # q^T and K^T: [co, t] = sum_c W[c, co]^T x^T[c, t]
                    for w_sb, dst in ((wq_sb, qT_sb), (wk_sb, kT_sb)):
                        for co in range(2):
                            ps = psum_p1.tile([128, 512], F32, tag="psA")
                            for ci in range(NCT):
                                nc.tensor.matmul(
                                    ps[:],
                                    w_sb[:, ci, ts(co, 128)],
                                    xbf[:, ci, :],
                                    start=(ci == 0),
                                    stop=(ci == NCT - 1),
                                )
                            nc.vector.tensor_copy(dst[:, co, ts(tj, 512)], ps[:])

                    # V: [t, co] = sum_c x^T[c, t]^T W_v^T[c, co]; aug layout
                    for tl in range(4):
                        tt = 4 * tj + tl
                        ps = psum_p1.tile([128, COC], F32, tag="psB")
                        for ci in range(NCT):
                            nc.tensor.matmul(
                                ps[:],
                                xbf[:, ci, ts(tl, 128)],
                                wv_sb[:, ci, :],
                                start=(ci == 0),
                                stop=(ci == NCT - 1),
                            )
                        nc.vector.tensor_copy(
                            v_sb[:, tt, :, 0:D],
                            ps[:].rearrange("p (h d) -> p h d", h=HPC),
                        )
                nc.vector.memset(v_sb[:, :, :, D], 1.0)

            # ---- phase 2: attention, S^T layout ----
            for m in range(4):
                nc.gpsimd.memset(masks[:, m, :], 0.0)
                nc.gpsimd.affine_select(
                    out=masks[:, m, :],
                    in_=masks[:, m, :],
                    pattern=[[1, 512]],
                    compare_op=mybir.AluOpType.is_ge,
                    fill=NEG,
                    base=-128 * m,
                    channel_multiplier=-1,
                )

            with (
                tc.tile_pool(name="p2psum_s", bufs=2, space="PSUM") as psum_s,
                tc.tile_pool(name="p2psum_o", bufs=2, space="PSUM") as psum_o,
            ):
                self_attention(
                    nc, psum_s, psum_o, work, qT_sb, kT_sb, v_sb, masks, otall
                )

            # ---- phase 3: exchange O^T across the 4 cores of this batch ----
            bounce_in = dram.tile([COC, T], BF16)
            bounce_out = dram.tile([C, T], BF16)
            for i in range(2):
                nc.sync.dma_start(out=bounce_in[ts(i, 128), :], in_=otall[:, i, :])
            nc.gpsimd.collective_compute(
                "AllGather",
                mybir.AluOpType.bypass,
                replica_groups=[[0, 1, 2, 3], [4, 5, 6, 7]],
                ins=[bounce_in.opt()],
                outs=[bounce_out.opt()],
            )
            nc.sync.dma_start(
                out=otfull[:], in_=bounce_out[:].rearrange("(a p) t -> p a t", p=128)
            )

            # ---- phase 4: output projection (bf16), oc-sharded ----
            wo_f32 = main.tile([128, NCT, COC], F32)
            nc.sync.dma_start(
                out=wo_f32[:], in_=woT_d.rearrange("(a p) t -> p a t", p=128)
            )
            for ci in range(NCT):
                nc.vector.tensor_copy(woT_bf[:, ci, :], wo_f32[:, ci, :])

            with tc.tile_pool(name="p4psum", bufs=4, space="PSUM") as psum_p4:
                for qt in range(T // 128):
                    ps = psum_p4.tile([128, COC], F32, tag="po")
                    for ci in range(NCT):
                        nc.tensor.matmul(
                            ps[:],
                            otfull[:, ci, ts(qt, 128)],
                            woT_bf[:, ci, :],
                            start=(ci == 0),
                            stop=(ci == NCT - 1),
                        )
                    ot = work.tile([128, COC], F32, tag="outst")
                    nc.vector.tensor_copy(ot[:], ps[:])
                    nc.sync.dma_start(out=out_d[ts(qt, 128), :], in_=ot[:])

    nc.compile()
    return nc


_NC_CACHE = None


def _get_nc():
    global _NC_CACHE
    if _NC_CACHE is None:
        _NC_CACHE = build_nc()
    return _NC_CACHE


def make_in_maps(x, Wq, Wk, Wv, Wo):
    x = np.asarray(x, dtype=np.float32)
    in_maps = []
    for c in range(N_CORES):
        b, g = c // 4, c % 4
        sl = slice(COC * g, COC * g + COC)
        in_maps.append(
            {
                "xT": np.ascontiguousarray(x[b].T),
                "wqT": np.ascontiguousarray(np.asarray(Wq)[sl, :].T),
                "wkT": np.ascontiguousarray(np.asarray(Wk)[sl, :].T),
                "wvT": np.ascontiguousarray(np.asarray(Wv)[sl, :].T),
                "woT": np.ascontiguousarray(np.asarray(Wo)[sl, :].T),
            }
        )
    return in_maps


def assemble(results):
    out = np.empty((B, T, C), dtype=np.float32)
    for c in range(N_CORES):
        b, g = c // 4, c % 4
        out[b, :, COC * g : COC * g + COC] = results[c]["out"]
    return out


def kernel(x, Wq, Wk, Wv, Wo):
    nc = _get_nc()
    in_maps = make_in_maps(x, Wq, Wk, Wv, Wo)
    res = run_bass_kernel_spmd(nc, in_maps, list(range(N_CORES)))
    return assemble(res.results)


if __name__ == "__main__":
    rng = np.random.default_rng(0)
    x = rng.standard_normal((B, T, C), dtype=np.float32)
    s = 1.0 / np.sqrt(C)
    ws = [
        rng.uniform(-s, s, size=(C, C)).astype(np.float32) for _ in range(4)
    ]
    out = kernel(x, *ws)
    print("kernel ran; out", out.shape, out.dtype)
